# revision 7
# baseline (speedup 1.0000x reference)
"""Trainium2 Bass kernel for out = x * exclusive_cumsum(x, axis=time).

Input x: [B=8, T=4096, D=1024] f32. Pure data parallel: batch element b -> core b.

Per-core algorithm (x_c: [T, D]), pair-interleaved for 8KB DMA descriptors:
  - T is split into 16 tiles of 256 rows; partition p of tile k holds rows
    2p and 2p+1 (even half at free cols [0,1024), odd at [1024,2048)), so
    every load/store descriptor covers 8KB of contiguous HBM (vs 4KB/2KB in
    the row-per-partition layout) -- fewer descriptors, higher sustained DMA.
  - Pool engine computes pair sums s = x_even + x_odd (f16 out); the PE then
    needs only s: a strict-upper-triangular 128x128 matmul gives the
    pair-exclusive prefix P[p] = sum_{p'<p} s[p'], and a selector matmul
    accumulates each tile's column total into one row of a group PSUM tile
    (2 groups of 8 tiles; group g's totals land at quadrant-aligned rows
    32g..32g+7 of a shared f16 totals tile, gap rows zeroed).  A carry
    matmul with lhsT = wcar[0:32g+i, :] (wcar[k,m]=1 iff k%32<8) adds the
    totals of tiles < k into every partition.
  - prefix_even = carry + P (all in PSUM f32); prefix_odd = prefix_even +
    x_even (exact f32 on DVE).  out_even = x_e * prefix_even,
    out_odd = x_o * prefix_odd.
  - Loads go on the sync HWDGE ring, the 3 tiny weight loads on the scalar
    (Activation) HWDGE ring so the first x descriptor generates immediately,
    and stores on the GpSimd SWDGE path.  Group 0's totals-prefix rows are
    copied out of PSUM early (rows <= i are final once selector matmul i
    retires; later selector matmuls add exact zeros there) so tile i+1's
    carry never waits on the full group-0 chain.

All bulk DMA moves linear 1MB tiles with 8KB/partition descriptors. PE
matmuls run in fp16 (1 cycle/row); accumulation stays fp32 in PSUM.
"""

import sys

sys.path.insert(0, "/opt/trn_rl_repo")

import numpy as np

B, T, D = 8, 4096, 1024
BLK = 128            # partitions per tile
PAIR = 2             # time rows per partition
TROW = BLK * PAIR    # 256 time rows per tile
NTILE = T // TROW    # 16
GRP = 8              # tiles per totals group
NGRP = NTILE // GRP  # 2
NCH = 2
CH = D // NCH        # 512, exactly one PSUM bank in f32

_CACHE = {}


def _weights(np_dtype=np.float16):
    wtri = np.triu(np.ones((BLK, BLK), dtype=np_dtype), 1)  # [k,m]=1 iff k<m
    # Selector: ones in column 64 only; wsel[:, 64-i : 64-i+GRP] has ones
    # exactly in slice-column i.
    wsel = np.zeros((BLK, BLK), dtype=np_dtype)
    wsel[:, 64] = 1.0
    # Carry weights: row k is all-ones iff it is a real totals row (k mod 32
    # < GRP); sliced to [0:32g+i, :] it sums exactly the totals of tiles < k
    # (totals rows sit at quadrant-aligned bases 32g; gap rows stay zero).
    k = np.arange(BLK)[:, None]
    wcar = ((k % 32) < GRP).astype(np_dtype) * np.ones((1, BLK), dtype=np_dtype)
    return wtri, wsel, wcar


def build_nc(t=T, d=D, nch=NCH, num_devices=B, early_copies=True):
    # early_copies: group-0 totals-prefix copies read finalized PSUM rows
    # while the accumulation group is still open. Verified correct on HW
    # (Tile orders copy_i between matmul_i and matmul_{i+1}; later matmuls
    # add exact zeros to rows <= i), but CoreSim forbids mid-group PSUM
    # reads, so a sim harness must build with early_copies=False.
    """Build the Bass module for one core's [t, d] shard."""
    import concourse.bass as bass
    import concourse.mybir as mybir
    import concourse.tile as tile
    from concourse import bacc

    f32 = mybir.dt.float32
    f16 = mybir.dt.float16
    ch = d // nch
    ntile = t // TROW
    ngrp = (ntile + GRP - 1) // GRP
    fw = PAIR * d        # free width of an interleaved tile
    assert t % TROW == 0 and d % nch == 0 and ch <= 512 and ntile <= 16

    nc = bacc.Bacc("TRN2", target_bir_lowering=False, debug=False,
                   num_devices=num_devices)
    x = nc.dram_tensor("x", [t, d], f32, kind="ExternalInput").ap()
    wtri = nc.dram_tensor("wtri", [BLK, BLK], f16, kind="ExternalInput").ap()
    wsel = nc.dram_tensor("wsel", [BLK, BLK], f16, kind="ExternalInput").ap()
    wcar = nc.dram_tensor("wcar", [BLK, BLK], f16, kind="ExternalInput").ap()
    out = nc.dram_tensor("out", [t, d], f32, kind="ExternalOutput").ap()

    def itile(ap, k):
        # [128, 2048] view of time rows [k*256, (k+1)*256): partition p
        # holds rows 2p (cols 0:1024) and 2p+1 (cols 1024:2048) -- one 8KB
        # contiguous HBM run per partition.
        return ap[k * TROW:(k + 1) * TROW, :].rearrange(
            "(p two) d -> p (two d)", two=PAIR)

    with tile.TileContext(nc) as tc:
        with (
            tc.tile_pool(name="wpool", bufs=1) as wpool,
            tc.tile_pool(name="xpool", bufs=10) as xpool,
            tc.tile_pool(name="spool", bufs=16) as spool,
            tc.tile_pool(name="tpool", bufs=1) as tpool,
            tc.tile_pool(name="mpool", bufs=3) as mpool,
            tc.tile_pool(name="opool", bufs=5) as opool,
            tc.tile_pool(name="ptot", bufs=1,
                         space=bass.MemorySpace.PSUM) as ptot,
            tc.tile_pool(name="pblk", bufs=3,
                         space=bass.MemorySpace.PSUM) as pblk,
        ):
            # First x load descriptor should generate ASAP on the sync ring;
            # the weight loads ride the (otherwise idle) scalar ring.
            xts = [None] * ntile
            s16s = [None] * ntile
            xts[0] = xpool.tile([BLK, fw], f32, tag="xt", name="xt0")
            nc.sync.dma_start(xts[0][:], itile(x, 0))

            wt = wpool.tile([BLK, BLK], f16, tag="wt")
            nc.scalar.dma_start(wt[:], wtri[:])
            ws = wpool.tile([BLK, BLK], f16, tag="ws")
            nc.scalar.dma_start(ws[:], wsel[:])
            wc = wpool.tile([BLK, BLK], f16, tag="wc")
            nc.scalar.dma_start(wc[:], wcar[:])

            totals = []
            for j in range(nch):
                tj = tpool.tile([BLK, ch], f16, tag=f"tots{j}",
                                name=f"totals{j}")
                # Gap rows (k%32 >= GRP) are read by carry matmuls with zero
                # weights; they must be 0.0 (not garbage NaN) since 0*NaN=NaN.
                nc.vector.memset(tj[:], 0.0)
                totals.append(tj)

            for g in range(ngrp):
                klo = g * GRP
                khi = min(klo + GRP, ntile)
                nk = khi - klo

                tot_psum = []
                for j in range(nch):
                    tp = ptot.tile([nk, ch], f32, tag=f"totg{j}",
                                   name=f"totg{g}_{j}")
                    tot_psum.append(tp)
                for i in range(nk):
                    k = klo + i
                    if xts[k] is None:
                        xts[k] = xpool.tile([BLK, fw], f32, tag="xt",
                                            name=f"xt{k}")
                        nc.sync.dma_start(xts[k][:], itile(x, k))
                    s16 = spool.tile([BLK, d], f16, tag="s16", name=f"s16_{k}")
                    nc.gpsimd.tensor_add(s16[:], xts[k][:, 0:d],
                                         xts[k][:, d:fw])
                    s16s[k] = s16
                    for j in range(nch):
                        jc = slice(j * ch, (j + 1) * ch)
                        nc.tensor.matmul(
                            tot_psum[j][:],
                            ws[:, 64 - i:64 - i + nk],  # slice-col i only
                            s16[:, jc],
                            start=(i == 0), stop=(i == nk - 1),
                        )
                        if early_copies and g == 0 and i < nk - 1:
                            # Early prefix copy: rows 0..i are final (later
                            # selector matmuls add exact zeros there), so
                            # tile i+1's carry unblocks without waiting for
                            # the whole group. Startup-critical group 0 only.
                            nc.vector.tensor_copy(
                                totals[j][0:i + 1, :],
                                tot_psum[j][0:i + 1, :])
                for j in range(nch):
                    nc.vector.tensor_copy(
                        totals[j][32 * g:32 * g + nk, :], tot_psum[j][:])

                for i in range(nk):
                    k = klo + i
                    kb = 32 * g + i  # totals rows covering tiles < k
                    ot = opool.tile([BLK, fw], f32, tag="out", name=f"ot{k}")
                    for j in range(nch):
                        jc = slice(j * ch, (j + 1) * ch)
                        oc = slice(d + j * ch, d + (j + 1) * ch)
                        ps = pblk.tile([BLK, ch], f32, tag=f"pb{j}",
                                       name=f"ps{k}_{j}")
                        nc.tensor.matmul(
                            ps[:], wt[:], s16s[k][:, jc],
                            start=True, stop=(kb == 0),
                        )
                        if kb > 0:
                            nc.tensor.matmul(
                                ps[:],
                                wc[0:kb, :],     # rows k%32<8 are ones
                                totals[j][0:kb, :],
                                start=False, stop=True,
                            )
                        # even rows: out_e = x_e * prefix
                        nc.vector.tensor_mul(ot[:, jc], xts[k][:, jc], ps[:])
                        # odd rows: prefix_o = prefix + x_e (exact f32)
                        tmp = mpool.tile([BLK, ch], f32, tag="tmp",
                                         name=f"tmp{k}_{j}")
                        nc.vector.tensor_add(tmp[:], ps[:], xts[k][:, jc])
                        nc.vector.tensor_mul(ot[:, oc], xts[k][:, oc], tmp[:])
                    # Full-width 1MB store (8KB/partition descriptors) from
                    # the otherwise idle GpSimd SWDGE path.
                    nc.gpsimd.dma_start(itile(out, k), ot[:])

    nc.compile()
    return nc


def kernel(x: np.ndarray) -> np.ndarray:
    from concourse.bass_utils import run_bass_kernel_spmd

    x = np.asarray(x, dtype=np.float32)
    assert x.shape == (B, T, D)
    key = "full"
    if key not in _CACHE:
        _CACHE[key] = build_nc()
    nc = _CACHE[key]

    wtri, wsel, wcar = _weights()
    in_maps = [
        {"x": np.ascontiguousarray(x[c]), "wtri": wtri, "wsel": wsel,
         "wcar": wcar}
        for c in range(B)
    ]
    res = run_bass_kernel_spmd(nc, in_maps, core_ids=list(range(B)))
    return np.stack([res.results[c]["out"] for c in range(B)], axis=0)


# revision 8
# speedup vs baseline: 1.1081x; 1.1081x over previous
"""Trainium2 Bass kernel for out = x * exclusive_cumsum(x, axis=time).

Input x: [B=8, T=4096, D=1024] f32. Pure data parallel: batch element b -> core b.

Per-core algorithm (x_c: [T, D], partition axis = time), group-pipelined:
  - T is split into 32 blocks of 128 rows, processed as 4 groups of 8 blocks.
  - Per block: one fp16 cast (ACT) feeds both passes below.
  - Totals: per block b = 8g+i, a colsum matmul with selector weights (ones in
    lhsT slice-column i) accumulates the block's column totals into row i of a
    group PSUM tile [8, 512] per 512-wide D chunk; one DVE copy per group drops
    them into rows [32g : 32g+8] of a shared fp16 totals tile [128, 512]
    (quadrant-aligned bases 0/32/64/96; gap rows stay zero via memset).
  - Per block: a strict-upper-triangular 128x128 matmul computes the
    within-block exclusive cumsum into PSUM (start=True); a second matmul with
    lhsT = wcar[0:32g+i, :] (wcar[k,m] = 1 iff k mod 32 < 8, so exactly the
    totals of blocks < b are summed; gap rows hit zero weights) adds the carry
    to every partition (start=False). DVE/ACT multiply f32 x by the f32 PSUM
    prefix; the result DMAs out.
  - Group g's compute starts as soon as its own totals copy lands, overlapping
    later groups' loads: the PE never waits on a global phase boundary.

Scheduling refinements over the plain version:
  - The first x block load is the first instruction on the sync HWDGE ring,
    and the three tiny weight loads ride the scalar (Activation) HWDGE ring,
    so bulk-load descriptors start generating ~2us earlier.
  - Output stores are full-width [128, 1024] (one per block instead of two
    512-wide chunks): half the store instructions/semaphores, 4KB HBM runs
    per descriptor. They issue from the (otherwise idle) GpSimd SWDGE path
    so they never head-of-line-block later loads on sync.

All bulk DMA is linear 512KB blocks. PE matmuls run in fp16 (1 cycle/row);
all accumulation stays fp32 in PSUM.
"""

import sys

sys.path.insert(0, "/opt/trn_rl_repo")

import numpy as np

B, T, D = 8, 4096, 1024
BLK = 128
NBLK = T // BLK      # 32
GRP = 8              # blocks per group
NGRP = NBLK // GRP   # 4
NCH = 2
CH = D // NCH        # 512, exactly one PSUM bank in f32

_CACHE = {}


def _weights(np_dtype=np.float16):
    wtri = np.triu(np.ones((BLK, BLK), dtype=np_dtype), 1)  # [k,m]=1 iff k<m
    # Selector: ones in column 64 only; wsel[:, 64-i : 72-i] has ones exactly
    # in slice-column i.
    wsel = np.zeros((BLK, BLK), dtype=np_dtype)
    wsel[:, 64] = 1.0
    # Carry weights: row k is all-ones iff it is a real totals row (k mod 32
    # < GRP); sliced to [0:32g+i, :] it sums exactly the totals of blocks < b.
    k = np.arange(BLK)[:, None]
    wcar = ((k % 32) < GRP).astype(np_dtype) * np.ones((1, BLK), dtype=np_dtype)
    return wtri, wsel, wcar


def build_nc(t=T, d=D, nch=NCH, num_devices=B, early_copies=True):
    # early_copies: group-0 totals-prefix copies read finalized PSUM rows
    # while the accumulation group is still open. Verified correct on HW
    # (Tile orders copy_i between matmul_i and matmul_{i+1}; later matmuls
    # add exact zeros to rows <= i), but CoreSim forbids mid-group PSUM
    # reads, so the sim harness builds with early_copies=False.
    """Build the Bass module for one core's [t, d] shard."""
    import concourse.bass as bass
    import concourse.mybir as mybir
    import concourse.tile as tile
    from concourse import bacc

    f32 = mybir.dt.float32
    f16 = mybir.dt.float16
    ch = d // nch
    nblk = t // BLK
    ngrp = (nblk + GRP - 1) // GRP
    assert t % BLK == 0 and d % nch == 0 and ch <= 512 and nblk <= 32

    nc = bacc.Bacc("TRN2", target_bir_lowering=False, debug=False,
                   num_devices=num_devices)
    x = nc.dram_tensor("x", [t, d], f32, kind="ExternalInput").ap()
    wtri = nc.dram_tensor("wtri", [BLK, BLK], f16, kind="ExternalInput").ap()
    wsel = nc.dram_tensor("wsel", [BLK, BLK], f16, kind="ExternalInput").ap()
    wcar = nc.dram_tensor("wcar", [BLK, BLK], f16, kind="ExternalInput").ap()
    out = nc.dram_tensor("out", [t, d], f32, kind="ExternalOutput").ap()

    with tile.TileContext(nc) as tc:
        with (
            tc.tile_pool(name="wpool", bufs=1) as wpool,
            tc.tile_pool(name="xpool", bufs=16) as xpool,
            tc.tile_pool(name="hpool", bufs=12) as hpool,
            tc.tile_pool(name="spool", bufs=1) as spool,
            tc.tile_pool(name="opool", bufs=8) as opool,
            tc.tile_pool(name="ptot", bufs=1,
                         space=bass.MemorySpace.PSUM) as ptot,
            tc.tile_pool(name="pblk", bufs=3,
                         space=bass.MemorySpace.PSUM) as pblk,
        ):
            # First x block load leads the sync ring; weight loads ride the
            # (otherwise idle) scalar HWDGE ring so bulk-load descriptor
            # generation starts immediately.
            xts_all = [None] * nblk
            xts_all[0] = xpool.tile([BLK, d], f32, tag="xt", name="xt0")
            nc.sync.dma_start(xts_all[0][:], x[0:BLK, :])

            wt = wpool.tile([BLK, BLK], f16, tag="wt")
            nc.scalar.dma_start(wt[:], wtri[:])
            ws = wpool.tile([BLK, BLK], f16, tag="ws")
            nc.scalar.dma_start(ws[:], wsel[:])
            wc = wpool.tile([BLK, BLK], f16, tag="wc")
            nc.scalar.dma_start(wc[:], wcar[:])

            totals = []
            for j in range(nch):
                tj = spool.tile([BLK, ch], f16, tag=f"tots{j}",
                                name=f"totals{j}")
                nc.vector.memset(tj[:], 0.0)
                totals.append(tj)

            for g in range(ngrp):
                blo = g * GRP
                bhi = min(blo + GRP, nblk)
                nb = bhi - blo

                xts, xas = [], []
                tot_psum = []
                for j in range(nch):
                    tp = ptot.tile([nb, ch], f32, tag=f"totg{j}",
                                   name=f"totg{g}_{j}")
                    tot_psum.append(tp)
                for i in range(nb):
                    b = blo + i
                    if xts_all[b] is None:
                        xts_all[b] = xpool.tile([BLK, d], f32, tag="xt",
                                                name=f"xt{b}")
                        nc.sync.dma_start(xts_all[b][:],
                                          x[b * BLK:(b + 1) * BLK, :])
                    xt = xts_all[b]
                    xts.append(xt)
                    xa = hpool.tile([BLK, d], f16, tag="xa", name=f"xa{b}")
                    nc.scalar.copy(xa[:], xt[:])
                    xas.append(xa)
                    for j in range(nch):
                        jc = slice(j * ch, (j + 1) * ch)
                        nc.tensor.matmul(
                            tot_psum[j][:],
                            ws[:, 64 - i:64 - i + nb],  # slice-col i only
                            xa[:, jc],
                            start=(i == 0), stop=(i == nb - 1),
                        )
                        if early_copies and g == 0 and i < nb - 1:
                            # Early prefix copy: rows 0..i are final (later
                            # selector matmuls add exact zeros there), so
                            # block i+1's carry unblocks without waiting for
                            # the whole group. Startup-critical group 0 only:
                            # extending this to all groups was measured SLOWER
                            # (DVE congestion + totals-tile WAR ping-pong).
                            nc.vector.tensor_copy(
                                totals[j][0:i + 1, :],
                                tot_psum[j][0:i + 1, :])
                for j in range(nch):
                    nc.vector.tensor_copy(
                        totals[j][32 * g:32 * g + nb, :], tot_psum[j][:])

                for i in range(nb):
                    b = blo + i
                    kb = 32 * g + i  # totals rows covering blocks < b
                    ot = opool.tile([BLK, d], f32, tag="out", name=f"ot{b}")
                    for j in range(nch):
                        jc = slice(j * ch, (j + 1) * ch)
                        ps = pblk.tile([BLK, ch], f32, tag=f"pb{j}",
                                       name=f"ps{b}_{j}")
                        nc.tensor.matmul(
                            ps[:], wt[:], xas[i][:, jc],
                            start=True, stop=(kb == 0),
                        )
                        if kb > 0:
                            nc.tensor.matmul(
                                ps[:],
                                wc[0:kb, :],         # rows k%32<8 are ones
                                totals[j][0:kb, :],
                                start=False, stop=True,
                            )
                        nc.any.tensor_mul(ot[:, jc], xts[i][:, jc],
                                          ps[:])
                    # Full-width 512KB store (4KB/partition descriptors):
                    # half the store instructions/semaphores vs per-chunk.
                    # GpSimd SWDGE keeps store descriptor generation off the
                    # sync ring so it never blocks later loads.
                    nc.gpsimd.dma_start(
                        out[b * BLK:(b + 1) * BLK, :], ot[:])

    nc.compile()
    return nc


def kernel(x: np.ndarray) -> np.ndarray:
    from concourse.bass_utils import run_bass_kernel_spmd

    x = np.asarray(x, dtype=np.float32)
    assert x.shape == (B, T, D)
    key = "full"
    if key not in _CACHE:
        _CACHE[key] = build_nc()
    nc = _CACHE[key]

    wtri, wsel, wcar = _weights()
    in_maps = [
        {"x": np.ascontiguousarray(x[c]), "wtri": wtri, "wsel": wsel,
         "wcar": wcar}
        for c in range(B)
    ]
    res = run_bass_kernel_spmd(nc, in_maps, core_ids=list(range(B)))
    return np.stack([res.results[c]["out"] for c in range(B)], axis=0)


# revision 9
# speedup vs baseline: 1.2028x; 1.0855x over previous
"""Trainium2 Bass kernel for out = x * exclusive_cumsum(x, axis=time).

Input x: [B=8, T=4096, D=1024] f32. Pure data parallel: batch element b -> core b.

Per-core algorithm (x_c: [T, D], partition axis = time), group-pipelined:
  - T is split into 32 blocks of 128 rows, processed as 4 groups of 8 blocks.
  - Per block: one fp16 cast (ACT) feeds both passes below.
  - Totals: per block b = 8g+i, a colsum matmul with selector weights (ones in
    lhsT slice-column i) accumulates the block's column totals into row i of a
    group PSUM tile [8, 512] per 512-wide D chunk; one DVE copy per group drops
    them into rows [32g : 32g+8] of a shared fp16 totals tile [128, 512]
    (quadrant-aligned bases 0/32/64/96; gap rows stay zero via memset).
  - Per block: a strict-upper-triangular 128x128 matmul computes the
    within-block exclusive cumsum into PSUM (start=True); a second matmul with
    lhsT = wcar[0:32g+i, :] (wcar[k,m] = 1 iff k mod 32 < 8, so exactly the
    totals of blocks < b are summed; gap rows hit zero weights) adds the carry
    to every partition (start=False). DVE/ACT multiply f32 x by the f32 PSUM
    prefix; the result DMAs out.
  - Group g's compute starts as soon as its own totals copy lands, overlapping
    later groups' loads: the PE never waits on a global phase boundary.

Scheduling refinements over the plain version:
  - The first x block load is the first instruction on the sync HWDGE ring,
    and the three tiny weight loads ride the scalar (Activation) HWDGE ring,
    so bulk-load descriptors start generating ~2us earlier.
  - Output stores are full-width [128, 1024] (one per block instead of two
    512-wide chunks): half the store instructions/semaphores, 4KB HBM runs
    per descriptor. They issue from the (otherwise idle) GpSimd SWDGE path
    so they never head-of-line-block later loads on sync.

All bulk DMA is linear 512KB blocks. PE matmuls run in fp16 (1 cycle/row);
all accumulation stays fp32 in PSUM.
"""

import sys

sys.path.insert(0, "/opt/trn_rl_repo")

import numpy as np

B, T, D = 8, 4096, 1024
BLK = 128
NBLK = T // BLK      # 32
GRP = 8              # blocks per group
NGRP = NBLK // GRP   # 4
NCH = 2
CH = D // NCH        # 512, exactly one PSUM bank in f32

_CACHE = {}


def _weights(np_dtype=np.float16):
    wtri = np.triu(np.ones((BLK, BLK), dtype=np_dtype), 1)  # [k,m]=1 iff k<m
    # Selector: ones in column 64 only; wsel[:, 64-i : 72-i] has ones exactly
    # in slice-column i.
    wsel = np.zeros((BLK, BLK), dtype=np_dtype)
    wsel[:, 64] = 1.0
    # Carry weights: row k is all-ones iff it is a real totals row (k mod 32
    # < GRP); sliced to [0:32g+i, :] it sums exactly the totals of blocks < b.
    k = np.arange(BLK)[:, None]
    wcar = ((k % 32) < GRP).astype(np_dtype) * np.ones((1, BLK), dtype=np_dtype)
    return wtri, wsel, wcar


def build_nc(t=T, d=D, nch=NCH, num_devices=B, early_copies=True):
    # early_copies: group-0 totals-prefix copies read finalized PSUM rows
    # while the accumulation group is still open. Verified correct on HW
    # (Tile orders copy_i between matmul_i and matmul_{i+1}; later matmuls
    # add exact zeros to rows <= i), but CoreSim forbids mid-group PSUM
    # reads, so the sim harness builds with early_copies=False.
    """Build the Bass module for one core's [t, d] shard."""
    import concourse.bass as bass
    import concourse.mybir as mybir
    import concourse.tile as tile
    from concourse import bacc

    f32 = mybir.dt.float32
    f16 = mybir.dt.float16
    ch = d // nch
    nblk = t // BLK
    ngrp = (nblk + GRP - 1) // GRP
    assert t % BLK == 0 and d % nch == 0 and ch <= 512 and nblk <= 32

    nc = bacc.Bacc("TRN2", target_bir_lowering=False, debug=False,
                   num_devices=num_devices)
    x = nc.dram_tensor("x", [t, d], f32, kind="ExternalInput").ap()
    wtri = nc.dram_tensor("wtri", [BLK, BLK], f16, kind="ExternalInput").ap()
    wsel = nc.dram_tensor("wsel", [BLK, BLK], f16, kind="ExternalInput").ap()
    wcar = nc.dram_tensor("wcar", [BLK, BLK], f16, kind="ExternalInput").ap()
    out = nc.dram_tensor("out", [t, d], f32, kind="ExternalOutput").ap()

    with tile.TileContext(nc) as tc:
        with (
            tc.tile_pool(name="wpool", bufs=1) as wpool,
            tc.tile_pool(name="xpool", bufs=16) as xpool,
            tc.tile_pool(name="hpool", bufs=12) as hpool,
            tc.tile_pool(name="spool", bufs=1) as spool,
            tc.tile_pool(name="opool", bufs=8) as opool,
            tc.tile_pool(name="ptot", bufs=1,
                         space=bass.MemorySpace.PSUM) as ptot,
            tc.tile_pool(name="pblk", bufs=3,
                         space=bass.MemorySpace.PSUM) as pblk,
        ):
            # First x block load leads the sync ring; weight loads ride the
            # (otherwise idle) scalar HWDGE ring so bulk-load descriptor
            # generation starts immediately.
            xts_all = [None] * nblk
            xts_all[0] = xpool.tile([BLK, d], f32, tag="xt", name="xt0")
            nc.sync.dma_start(xts_all[0][:], x[0:BLK, :])

            wt = wpool.tile([BLK, BLK], f16, tag="wt")
            nc.scalar.dma_start(wt[:], wtri[:])
            ws = wpool.tile([BLK, BLK], f16, tag="ws")
            nc.scalar.dma_start(ws[:], wsel[:])
            wc = wpool.tile([BLK, BLK], f16, tag="wc")
            nc.scalar.dma_start(wc[:], wcar[:])

            totals = []
            for j in range(nch):
                tj = spool.tile([BLK, ch], f16, tag=f"tots{j}",
                                name=f"totals{j}")
                nc.vector.memset(tj[:], 0.0)
                totals.append(tj)

            for g in range(ngrp):
                blo = g * GRP
                bhi = min(blo + GRP, nblk)
                nb = bhi - blo

                xts, xas = [], []
                tot_psum = []
                for j in range(nch):
                    tp = ptot.tile([nb, ch], f32, tag=f"totg{j}",
                                   name=f"totg{g}_{j}")
                    tot_psum.append(tp)
                for i in range(nb):
                    b = blo + i
                    if xts_all[b] is None:
                        xts_all[b] = xpool.tile([BLK, d], f32, tag="xt",
                                                name=f"xt{b}")
                        nc.sync.dma_start(xts_all[b][:],
                                          x[b * BLK:(b + 1) * BLK, :])
                    xt = xts_all[b]
                    xts.append(xt)
                    xa = hpool.tile([BLK, d], f16, tag="xa", name=f"xa{b}")
                    nc.scalar.copy(xa[:], xt[:])
                    xas.append(xa)
                    for j in range(nch):
                        jc = slice(j * ch, (j + 1) * ch)
                        nc.tensor.matmul(
                            tot_psum[j][:],
                            ws[:, 64 - i:64 - i + nb],  # slice-col i only
                            xa[:, jc],
                            start=(i == 0), stop=(i == nb - 1),
                        )
                        if early_copies and g == 0 and i < nb - 1:
                            # Early prefix copy: rows 0..i are final (later
                            # selector matmuls add exact zeros there), so
                            # block i+1's carry unblocks without waiting for
                            # the whole group. Startup-critical group 0 only:
                            # extending this to all groups was measured SLOWER
                            # (DVE congestion + totals-tile WAR ping-pong).
                            nc.vector.tensor_copy(
                                totals[j][0:i + 1, :],
                                tot_psum[j][0:i + 1, :])
                for j in range(nch):
                    nc.vector.tensor_copy(
                        totals[j][32 * g:32 * g + nb, :], tot_psum[j][:])

                for i in range(nb):
                    b = blo + i
                    kb = 32 * g + i  # totals rows covering blocks < b
                    ot = opool.tile([BLK, d], f32, tag="out", name=f"ot{b}")
                    for j in range(nch):
                        jc = slice(j * ch, (j + 1) * ch)
                        ps = pblk.tile([BLK, ch], f32, tag=f"pb{j}",
                                       name=f"ps{b}_{j}")
                        nc.tensor.matmul(
                            ps[:], wt[:], xas[i][:, jc],
                            start=True, stop=(kb == 0),
                        )
                        if kb > 0:
                            nc.tensor.matmul(
                                ps[:],
                                wc[0:kb, :],         # rows k%32<8 are ones
                                totals[j][0:kb, :],
                                start=False, stop=True,
                            )
                        nc.any.tensor_mul(ot[:, jc], xts[i][:, jc],
                                          ps[:])
                        # Stores issue from the (otherwise idle) GpSimd
                        # sequencer so they never head-of-line-block later
                        # loads on sync; per-chunk (2KB/partition rows --
                        # larger store descriptors make DMA engine 15 a
                        # measured ~10% straggler) so each starts as soon as
                        # its multiply lands.
                        nc.gpsimd.dma_start(
                            out[b * BLK:(b + 1) * BLK, jc], ot[:, jc])

    nc.compile()
    return nc


def kernel(x: np.ndarray) -> np.ndarray:
    from concourse.bass_utils import run_bass_kernel_spmd

    x = np.asarray(x, dtype=np.float32)
    assert x.shape == (B, T, D)
    key = "full"
    if key not in _CACHE:
        _CACHE[key] = build_nc()
    nc = _CACHE[key]

    wtri, wsel, wcar = _weights()
    in_maps = [
        {"x": np.ascontiguousarray(x[c]), "wtri": wtri, "wsel": wsel,
         "wcar": wcar}
        for c in range(B)
    ]
    res = run_bass_kernel_spmd(nc, in_maps, core_ids=list(range(B)))
    return np.stack([res.results[c]["out"] for c in range(B)], axis=0)
